# revision 25
# baseline (speedup 1.0000x reference)
"""Trainium2 Bass kernel for the cross-head MultiHeadAttention module.

Reference computation (per row r of x flattened to (N*L, E)):
    q = x @ Wq; k = x @ Wk; v = x @ Wv           (E = 1024, H = 16, D = 64)
    energy[r, i, j] = sum_d q[r,i,d] * k[r,j,d]  (cross-head, per position)
    attn = softmax(energy / 32, axis=j)
    out[r, i, :] = sum_j attn[r,i,j] * v[r,j,:]
    y = out.reshape(R, E) @ Wo + bo

Distribution: data-parallel over rows (N*L = 16384 -> 2048 rows/core x 8).

Per-core design (all big matmuls in bf16 on the PE array), v3 "dense":
  *  Q/K projections run transposed (features on partitions, rows free);
     V runs natural (rows on partitions, features free). All three round-trip
     through DRAM to be re-read in attention-friendly layouts with >=128B
     contiguous runs on both DMA sides.
  *  Rows are processed in pairs (pi, pi + RC/2).  Energy: ONE matmul per
     pair: lhsT = qd2b[:, :, pi] — a [128, 32] block-diagonal slab (row pi's
     Q^T on partitions 0:64 x cols 0:16, row pi+RC/2's on 64:128 x 16:32,
     zeros elsewhere, zeroed once at startup and never rewritten); rhs =
     kht2[:, :, pi] ([128, 16]: both rows' K^T stacked).  Out: a dense
     [32, 16] block ep[32b:32b+32, s, :] — a 256-row psum bank, so softmax
     runs on fully dense [128, 512] tiles with no padding.
  *  softmax: exp (no max-subtraction: energies ~N(0, 1/16)), row-sum,
     reciprocal, scale+cast-to-bf16, then one 32x32-block vector transpose
     per bank flips each row's A to A^T in place.
  *  A@V: ONE matmul per row pair (b, t): lhsT = vd[32b:32b+32, slot, :] — a
     [32, 128] block-diagonal V slab (built by DMA into a once-zeroed tile),
     rhs = att[32b:32b+32, t, :], out = avp[64w+d, t, q] for both rows.
  *  Extraction: 4 strided copies per (bank, b) move avp psum into
     oft2[64*(q%2)+d, q//2, r] (vector/scalar; half the copies shift
     partitions by +-64, which the engines support).
  *  y^T: full-width Wo matmuls: lhsT = preloaded Wo slab [128, 128] (two
     head-chunks on partitions), rhs = oft2[:, g, :] contiguous, 8 chunks
     accumulated in psum; + bo; DMA out.  Output columns are in natural row
     order (no host-side permutation).
  *  Passes are software-pipelined: iteration `it` issues projections +
     staging round-trips for pass `it` and attention + output for pass
     `it-1`, so the PE alternates projection and attention work while DMA
     round-trips and softmax run under it.
"""

import numpy as np
import ml_dtypes

import concourse.bass as bass
from concourse import bacc
import concourse.tile as tile
from concourse import mybir
from concourse.bass_utils import run_bass_kernel_spmd

F32 = mybir.dt.float32
BF16 = mybir.dt.bfloat16
AF = mybir.ActivationFunctionType
ALU = mybir.AluOpType
AX = mybir.AxisListType

E = 1024
H = 16
D = 64
NCORE = 8


def build_nc(R, RC):
    """Per-core kernel program: R rows total, processed in passes of RC."""
    NP = R // RC          # passes
    NBK = RC // 256       # dense energy banks per pass (256 rows each)
    PH = RC // 2          # row pairs per pass

    nc = bacc.Bacc("TRN2", target_bir_lowering=False, debug=False)

    xt = nc.dram_tensor("xt", [E, R], BF16, kind="ExternalInput")
    wq = nc.dram_tensor("wq", [E, E], BF16, kind="ExternalInput")
    wk = nc.dram_tensor("wk", [E, E], BF16, kind="ExternalInput")
    wv = nc.dram_tensor("wv", [E, E], BF16, kind="ExternalInput")
    wo = nc.dram_tensor("wo", [E, E], BF16, kind="ExternalInput")
    bo = nc.dram_tensor("bo", [1, E], F32, kind="ExternalInput")
    yt = nc.dram_tensor("yt", [E, R], F32, kind="ExternalOutput")

    with tile.TileContext(nc) as tc:
        with (
            tc.tile_pool(name="wpool", bufs=1) as wpool,      # persistent
            tc.tile_pool(name="xpool", bufs=2) as xpool,      # xt chunks
            tc.tile_pool(name="spool", bufs=1) as spool,      # q/k/v staging
            tc.tile_pool(name="hpool", bufs=1) as hpool,      # attn operands
            tc.tile_pool(name="apool", bufs=2) as apool,      # softmax temps
            tc.tile_pool(name="opool", bufs=1) as opool,      # oft2
            tc.tile_pool(name="ypool", bufs=3) as ypool,      # y staging
            tc.tile_pool(name="dram", bufs=2, space="DRAM") as dpool,
            tc.tile_pool(name="pproj", bufs=2, space="PSUM") as pproj,
            tc.tile_pool(name="pe", bufs=2, space="PSUM") as pe_pool,
            tc.tile_pool(name="pav", bufs=1, space="PSUM") as pav,
        ):
            # ---- persistent loads ----
            wq_sb = wpool.tile([128, 8, E], BF16, tag="wq")
            wk_sb = wpool.tile([128, 8, E], BF16, tag="wk")
            wv_sb = wpool.tile([128, 8, E], BF16, tag="wv")
            wo_sb = wpool.tile([128, 8, E], BF16, tag="wo")
            nc.sync.dma_start(wq_sb[:], wq.rearrange("(c p) e -> p c e", p=128))
            nc.sync.dma_start(wk_sb[:], wk.rearrange("(c p) e -> p c e", p=128))
            nc.sync.dma_start(wv_sb[:], wv.rearrange("(c p) e -> p c e", p=128))
            # wo_sb[64s+d, g, e'] = Wo[64*(2g+s)+d, e']: head i -> (s=i%2, g=i//2)
            nc.sync.dma_start(wo_sb[:], wo.rearrange("(g p) e -> p g e", p=128))
            bo_sb = wpool.tile([128, 8], F32, tag="bo")
            nc.sync.dma_start(bo_sb[:], bo.rearrange("o (t p) -> p t o", p=128).squeeze(-1))

            # block-diagonal operand tiles: zero blocks are memset once and
            # never rewritten (per-pass DMAs touch only the data blocks).
            qd2b = wpool.tile([128, 32, PH], BF16, tag="qd2b")
            nc.vector.memset(qd2b[0:64, 16:32, :], 0.0)
            nc.vector.memset(qd2b[64:128, 0:16, :], 0.0)
            kht2 = wpool.tile([128, 16, PH], BF16, tag="kht2")
            vd = wpool.tile([128, NBK * 32, 128], BF16, tag="vd")
            nc.vector.memset(vd[:], 0.0)

            oft2 = opool.tile([128, 8, RC], BF16, tag="oft2")

            for it in range(NP + 1):
                if it < NP:
                    p, r0 = it, it * RC
                    # ---- x chunk ----
                    xtc = xpool.tile([128, 8, RC], BF16, tag="xtc")
                    nc.sync.dma_start(
                        xtc[:],
                        xt.rearrange("(c p) r -> p c r", p=128)[:, :, r0:r0 + RC],
                    )

                    # ---- Q/K projections (transposed) + stage-out ----
                    stage_of = {}
                    for name, w_sb in (("q", wq_sb), ("k", wk_sb)):
                        stg = spool.tile([128, 8, RC], BF16, tag=f"stg_{name}")
                        for et in range(8):
                            ps = pproj.tile([128, RC], F32, tag="proj")
                            for c in range(8):
                                nc.tensor.matmul(
                                    ps[:],
                                    w_sb[:, c, et * 128:(et + 1) * 128],
                                    xtc[:, c, :],
                                    start=(c == 0),
                                    stop=(c == 7),
                                )
                            if et % 2 == 0:
                                nc.vector.tensor_copy(stg[:, et, :], ps[:])
                            else:
                                nc.scalar.copy(stg[:, et, :], ps[:])
                        dt = dpool.tile([E, RC], BF16, tag=f"dram_{name}")
                        nc.sync.dma_start(
                            dt[:].rearrange("(t q) r -> q t r", q=128), stg[:]
                        )
                        stage_of[name] = dt

                    # ---- V projection (natural row-major) + roundtrip ----
                    vstg = spool.tile([128, RC // 128, E], BF16, tag="stg_v")
                    for rc_ in range(RC // 128):
                        for h2 in range(2):
                            ps = pproj.tile([128, 512], F32, tag="proj")
                            for c in range(8):
                                nc.tensor.matmul(
                                    ps[:],
                                    xtc[:, c, rc_ * 128:(rc_ + 1) * 128],
                                    wv_sb[:, c, h2 * 512:(h2 + 1) * 512],
                                    start=(c == 0),
                                    stop=(c == 7),
                                )
                            if h2 == 0:
                                nc.vector.tensor_copy(
                                    vstg[:, rc_, 0:512], ps[:])
                            else:
                                nc.scalar.copy(
                                    vstg[:, rc_, 512:1024], ps[:])
                    v2d = dpool.tile([RC, E], BF16, tag="dram_v")
                    nc.sync.dma_start(
                        v2d[:].rearrange("(rc p) e -> p rc e", p=128), vstg[:]
                    )
                    stage_of["v"] = v2d

                if it >= 1:
                    p = it - 1
                    # ---- attention for pass p ----
                    # both banks' energies back-to-back into one 2-bank psum
                    # tile; each bank's softmax overlaps the next energy block
                    atts = []
                    for B in range(NBK):
                        ep = pe_pool.tile([128, 32, 16], F32, tag="ep")
                        for lam in range(128):
                            pi = 128 * B + lam
                            b, s = (lam // 16) % 4, 2 * (lam % 16) + lam // 64
                            nc.tensor.matmul(
                                ep[32 * b:32 * b + 32, s, :],
                                qd2b[:, :, pi],
                                kht2[:, :, pi],
                                start=True,
                                stop=True,
                                tile_position=(0, 32 * b),
                            )
                        # dense softmax over the 256-row bank
                        ex = apool.tile([128, 32, 16], F32, tag="ex")
                        nc.scalar.activation(ex[:], ep[:], AF.Exp)
                        sm = apool.tile([128, 32], F32, tag="sm")
                        nc.vector.reduce_sum(sm[:], ex[:], axis=AX.X)
                        rcp = apool.tile([128, 32], F32, tag="rcp")
                        nc.vector.reciprocal(rcp[:], sm[:])
                        at = apool.tile([128, 32, 16], BF16, tag="at")
                        nc.vector.tensor_tensor(
                            at[:], ex[:],
                            rcp[:, :, None].to_broadcast([128, 32, 16]),
                            ALU.mult,
                        )
                        att = apool.tile([128, 512], BF16, tag="att")
                        nc.vector.transpose(
                            att[:], at[:].rearrange("p a b -> p (a b)"))
                        atts.append(att)

                    # A @ V per bank: all four b-bands into one 4-bank
                    # psum tile, then 4 merged extraction copies
                    dstx = oft2[:].rearrange(
                        "p g (h Bk wc) -> p g h Bk wc", h=2, Bk=NBK)
                    for B in range(NBK):
                        att = atts[B]
                        avp = pav.tile([128, 4, 32, 16], F32, tag="avp")
                        for b in range(4):
                            for t in range(32):
                                nc.tensor.matmul(
                                    avp[:, b, t, :],
                                    vd[32 * b:32 * b + 32, 32 * B + t, :],
                                    att[32 * b:32 * b + 32,
                                        16 * t:16 * t + 16],
                                    start=True,
                                    stop=True,
                                    tile_position=(32 * b, 0),
                                )
                        # avp[64w+d, b, 2m+rho, q] -> oft2[64(q%2)+d, q//2,
                        #   256rho + 128B + 64w + 16b + m]
                        srcx = avp[:].rearrange(
                            "p b (m r) (g s) -> p g r (b m) s", r=2, s=2)
                        for w in range(2):
                            for sg in range(2):
                                src = srcx[64 * w:64 * w + 64, :, :, :, sg]
                                dst = dstx[64 * sg:64 * sg + 64,
                                           :, :, B, 64 * w:64 * w + 64]
                                if (w + sg + B) % 2 == 0:
                                    nc.vector.tensor_copy(dst, src)
                                else:
                                    nc.scalar.copy(dst, src)

                    # ---- y^T = Wo^T-chunks @ oft2, + bo ----
                    for c in range(8):
                        # rotates through the ep buffers (attention is done
                        # with them by now) -> double-buffered Wo psum at no
                        # extra bank cost
                        ytp = pe_pool.tile([128, RC], F32, tag="ep")
                        for g in range(8):
                            nc.tensor.matmul(
                                ytp[:],
                                wo_sb[:, g, 128 * c:128 * c + 128],
                                oft2[:, g, :],
                                start=(g == 0),
                                stop=(g == 7),
                            )
                        ys = ypool.tile([128, RC], F32, tag="ys")
                        nc.vector.tensor_scalar(
                            ys[:], ytp[:], bo_sb[:, c:c + 1], None,
                            op0=ALU.add,
                        )
                        nc.sync.dma_start(
                            yt.rearrange("(t q) r -> q t r", q=128)[
                                :, c, p * RC:(p + 1) * RC
                            ],
                            ys[:],
                        )

                if it < NP:
                    # ---- readbacks for pass `it` (issued after pass it-1's
                    # attention so the shared bufs=1 operand tiles are free;
                    # the DMAs run under Wo(it-1) + projections(it+1)) ----
                    qsrc = stage_of["q"][:].rearrange(
                        "(q d) (h pi) -> h d q pi", q=H, h=2
                    )
                    nc.sync.dma_start(qd2b[0:64, 0:16, :], qsrc[0])
                    nc.sync.dma_start(qd2b[64:128, 16:32, :], qsrc[1])
                    ksrc = stage_of["k"][:].rearrange(
                        "(q d) (h pi) -> h d q pi", q=H, h=2
                    )
                    nc.sync.dma_start(kht2[0:64, :, :], ksrc[0])
                    nc.sync.dma_start(kht2[64:128, :, :], ksrc[1])
                    # vd[32b+16w+j, 32B+2a+rho, 64w+d] =
                    #   V[r = 256rho + 128B + 8a + 4w + b, (j, d)]
                    vsrc = stage_of["v"][:].rearrange(
                        "(h B w b m) (j d) -> h w B b j m d",
                        h=2, B=NBK, w=2, b=4, m=16, j=16,
                    )
                    for w in range(2):
                        for B in range(NBK):
                            for b in range(4):
                                for rho in range(2):
                                    nc.sync.dma_start(
                                        vd[32 * b + 16 * w:
                                           32 * b + 16 * w + 16,
                                           32 * B + rho:32 * B + 32:2,
                                           64 * w:64 * w + 64],
                                        vsrc[rho, w, B, b],
                                    )

    nc.finalize()
    return nc


_CACHE = {}


def _get_nc(R, RC):
    key = (R, RC)
    if key not in _CACHE:
        _CACHE[key] = build_nc(R, RC)
    return _CACHE[key]


def run_cores(x2d, Wq, Wk, Wv, Wo, bo_v, R=None, RC=512, cores=None,
              **run_kwargs):
    """x2d: (ROWS, E) fp32.  Returns (ROWS, E) fp32."""
    ROWS = x2d.shape[0]
    if cores is None:
        cores = list(range(NCORE))
    n = len(cores)
    if R is None:
        R = ROWS // n
    assert R * n == ROWS
    nc = _get_nc(R, RC)

    bf = ml_dtypes.bfloat16
    scale = 1.0 / np.sqrt(np.sqrt(float(E)))  # fold E**-0.5 into both Wq, Wk
    wq_b = (Wq.astype(np.float64) * scale).astype(bf)
    wk_b = (Wk.astype(np.float64) * scale).astype(bf)
    wv_b = Wv.astype(bf)
    wo_b = Wo.astype(bf)
    bo_in = bo_v.reshape(1, E).astype(np.float32)

    in_maps = []
    for ci in range(n):
        xs = x2d[ci * R:(ci + 1) * R].T  # (E, R)
        in_maps.append({
            "xt": np.ascontiguousarray(xs).astype(bf),
            "wq": wq_b, "wk": wk_b, "wv": wv_b, "wo": wo_b, "bo": bo_in,
        })
    res = run_bass_kernel_spmd(nc, in_maps, core_ids=cores, **run_kwargs)
    out = np.empty((ROWS, E), dtype=np.float32)
    for ci in range(n):
        ytd = res.results[ci]["yt"]  # (E, R), columns in natural row order
        out[ci * R:(ci + 1) * R] = ytd.T
    if run_kwargs.get("trace"):
        return out, res
    return out


def kernel(x, Wq, Wk, Wv, Wo, bo):
    x = np.asarray(x, dtype=np.float32)
    N, L, _ = x.shape
    y = run_cores(
        x.reshape(N * L, E),
        np.asarray(Wq, np.float32), np.asarray(Wk, np.float32),
        np.asarray(Wv, np.float32), np.asarray(Wo, np.float32),
        np.asarray(bo, np.float32),
    )
    return y.reshape(N, L, E)


# revision 26
# speedup vs baseline: 1.1539x; 1.1539x over previous
"""Trainium2 Bass kernel for the cross-head MultiHeadAttention module.

Reference computation (per row r of x flattened to (N*L, E)):
    q = x @ Wq; k = x @ Wk; v = x @ Wv           (E = 1024, H = 16, D = 64)
    energy[r, i, j] = sum_d q[r,i,d] * k[r,j,d]  (cross-head, per position)
    attn = softmax(energy / 32, axis=j)
    out[r, i, :] = sum_j attn[r,i,j] * v[r,j,:]
    y = out.reshape(R, E) @ Wo + bo

Distribution: data-parallel over rows (N*L = 16384 -> 2048 rows/core x 8).

Per-core design (all big matmuls in bf16 on the PE array), v3 "dense":
  *  Q/K projections run transposed (features on partitions, rows free);
     V runs natural (rows on partitions, features free). All three round-trip
     through DRAM to be re-read in attention-friendly layouts with >=128B
     contiguous runs on both DMA sides.
  *  Rows are processed in pairs (pi, pi + RC/2).  Energy: ONE matmul per
     pair: lhsT = qd2b[:, :, pi] — a [128, 32] block-diagonal slab (row pi's
     Q^T on partitions 0:64 x cols 0:16, row pi+RC/2's on 64:128 x 16:32,
     zeros elsewhere, zeroed once at startup and never rewritten); rhs =
     kht2[:, :, pi] ([128, 16]: both rows' K^T stacked).  Out: a dense
     [32, 16] block ep[32b:32b+32, s, :] — a 256-row psum bank, so softmax
     runs on fully dense [128, 512] tiles with no padding.
  *  softmax: exp (no max-subtraction: energies ~N(0, 1/16)), row-sum,
     reciprocal, scale+cast-to-bf16, then one 32x32-block vector transpose
     per bank flips each row's A to A^T in place.
  *  A@V: ONE matmul per row pair (b, t): lhsT = vd[32b:32b+32, slot, :] — a
     [32, 128] block-diagonal V slab (built by DMA into a once-zeroed tile),
     rhs = att[32b:32b+32, t, :], out = avp[64w+d, t, q] for both rows.
  *  Extraction: 4 strided copies per (bank, b) move avp psum into
     oft2[64*(q%2)+d, q//2, r] (vector/scalar; half the copies shift
     partitions by +-64, which the engines support).
  *  y^T: full-width Wo matmuls: lhsT = preloaded Wo slab [128, 128] (two
     head-chunks on partitions), rhs = oft2[:, g, :] contiguous, 8 chunks
     accumulated in psum; + bo; DMA out.  Output columns are in natural row
     order (no host-side permutation).
  *  Passes are software-pipelined: iteration `it` issues projections +
     staging round-trips for pass `it` and attention + output for pass
     `it-1`, so the PE alternates projection and attention work while DMA
     round-trips and softmax run under it.
"""

import numpy as np
import ml_dtypes

import concourse.bass as bass
from concourse import bacc
import concourse.tile as tile
from concourse import mybir
from concourse.bass_utils import run_bass_kernel_spmd

F32 = mybir.dt.float32
BF16 = mybir.dt.bfloat16
AF = mybir.ActivationFunctionType
ALU = mybir.AluOpType
AX = mybir.AxisListType

E = 1024
H = 16
D = 64
NCORE = 8


def build_nc(R, RC):
    """Per-core kernel program: R rows total, processed in passes of RC."""
    NP = R // RC          # passes
    NBK = RC // 256       # dense energy banks per pass (256 rows each)
    PH = RC // 2          # row pairs per pass

    nc = bacc.Bacc("TRN2", target_bir_lowering=False, debug=False)

    xt = nc.dram_tensor("xt", [E, R], BF16, kind="ExternalInput")
    wq = nc.dram_tensor("wq", [E, E], BF16, kind="ExternalInput")
    wk = nc.dram_tensor("wk", [E, E], BF16, kind="ExternalInput")
    wv = nc.dram_tensor("wv", [E, E], BF16, kind="ExternalInput")
    wo = nc.dram_tensor("wo", [E, E], BF16, kind="ExternalInput")
    bo = nc.dram_tensor("bo", [1, E], F32, kind="ExternalInput")
    yt = nc.dram_tensor("yt", [E, R], F32, kind="ExternalOutput")

    with tile.TileContext(nc) as tc:
        with (
            tc.tile_pool(name="wpool", bufs=1) as wpool,      # persistent
            tc.tile_pool(name="xpool", bufs=2) as xpool,      # xt chunks
            tc.tile_pool(name="spool", bufs=1) as spool,      # q/k/v staging
            tc.tile_pool(name="hpool", bufs=1) as hpool,      # attn operands
            tc.tile_pool(name="apool", bufs=2) as apool,      # softmax temps
            tc.tile_pool(name="opool", bufs=1) as opool,      # oft2
            tc.tile_pool(name="ypool", bufs=3) as ypool,      # y staging
            tc.tile_pool(name="dram", bufs=2, space="DRAM") as dpool,
            tc.tile_pool(name="pproj", bufs=2, space="PSUM") as pproj,
            tc.tile_pool(name="pe", bufs=2, space="PSUM") as pe_pool,
            tc.tile_pool(name="pav", bufs=1, space="PSUM") as pav,
        ):
            # ---- persistent loads ----
            wq_sb = wpool.tile([128, 8, E], BF16, tag="wq")
            wk_sb = wpool.tile([128, 8, E], BF16, tag="wk")
            wv_sb = wpool.tile([128, 8, E], BF16, tag="wv")
            wo_sb = wpool.tile([128, 8, E], BF16, tag="wo")
            nc.sync.dma_start(wq_sb[:], wq.rearrange("(c p) e -> p c e", p=128))
            nc.sync.dma_start(wk_sb[:], wk.rearrange("(c p) e -> p c e", p=128))
            nc.sync.dma_start(wv_sb[:], wv.rearrange("(c p) e -> p c e", p=128))
            # wo_sb[64s+d, g, e'] = Wo[64*(2g+s)+d, e']: head i -> (s=i%2, g=i//2)
            nc.sync.dma_start(wo_sb[:], wo.rearrange("(g p) e -> p g e", p=128))
            bo_sb = wpool.tile([128, 8], F32, tag="bo")
            nc.sync.dma_start(bo_sb[:], bo.rearrange("o (t p) -> p t o", p=128).squeeze(-1))

            # block-diagonal operand tiles: zero blocks are memset once and
            # never rewritten (per-pass DMAs touch only the data blocks).
            qd2b = wpool.tile([128, 32, PH], BF16, tag="qd2b")
            nc.vector.memset(qd2b[0:64, 16:32, :], 0.0)
            nc.vector.memset(qd2b[64:128, 0:16, :], 0.0)
            kht2 = wpool.tile([128, 16, PH], BF16, tag="kht2")
            vd = wpool.tile([128, NBK * 32, 128], BF16, tag="vd")
            nc.vector.memset(vd[:], 0.0)

            oft2 = opool.tile([128, 8, RC], BF16, tag="oft2")

            for it in range(NP + 1):
                if it < NP:
                    p, r0 = it, it * RC
                    # ---- x chunk ----
                    xtc = xpool.tile([128, 8, RC], BF16, tag="xtc")
                    nc.sync.dma_start(
                        xtc[:],
                        xt.rearrange("(c p) r -> p c r", p=128)[:, :, r0:r0 + RC],
                    )

                    # ---- Q/K projections (transposed) + stage-out ----
                    stage_of = {}
                    for name, w_sb in (("q", wq_sb), ("k", wk_sb)):
                        stg = spool.tile([128, 8, RC], BF16, tag=f"stg_{name}")
                        for et in range(8):
                            ps = pproj.tile([128, RC], F32, tag="proj")
                            for c in range(8):
                                nc.tensor.matmul(
                                    ps[:],
                                    w_sb[:, c, et * 128:(et + 1) * 128],
                                    xtc[:, c, :],
                                    start=(c == 0),
                                    stop=(c == 7),
                                )
                            if et % 2 == 0:
                                nc.vector.tensor_copy(stg[:, et, :], ps[:])
                            else:
                                nc.scalar.copy(stg[:, et, :], ps[:])
                        dt = dpool.tile([E, RC], BF16, tag=f"dram_{name}")
                        nc.sync.dma_start(
                            dt[:].rearrange("(t q) r -> q t r", q=128), stg[:]
                        )
                        stage_of[name] = dt

                    # ---- V projection (natural row-major) + roundtrip ----
                    vstg = spool.tile([128, RC // 128, E], BF16, tag="stg_v")
                    for rc_ in range(RC // 128):
                        for h2 in range(2):
                            ps = pproj.tile([128, 512], F32, tag="proj")
                            for c in range(8):
                                nc.tensor.matmul(
                                    ps[:],
                                    xtc[:, c, rc_ * 128:(rc_ + 1) * 128],
                                    wv_sb[:, c, h2 * 512:(h2 + 1) * 512],
                                    start=(c == 0),
                                    stop=(c == 7),
                                )
                            if h2 == 0:
                                nc.vector.tensor_copy(
                                    vstg[:, rc_, 0:512], ps[:])
                            else:
                                nc.scalar.copy(
                                    vstg[:, rc_, 512:1024], ps[:])
                    v2d = dpool.tile([RC, E], BF16, tag="dram_v")
                    nc.sync.dma_start(
                        v2d[:].rearrange("(rc p) e -> p rc e", p=128), vstg[:]
                    )
                    stage_of["v"] = v2d

                if it >= 1:
                    p = it - 1
                    # ---- attention for pass p ----
                    # both banks' energies back-to-back into one 2-bank psum
                    # tile; each bank's softmax overlaps the next energy block
                    atts = []
                    for B in range(NBK):
                        ep = pe_pool.tile([128, 32, 16], F32, tag="ep")
                        for lam in range(128):
                            pi = 128 * B + lam
                            b, s = (lam // 16) % 4, 2 * (lam % 16) + lam // 64
                            nc.tensor.matmul(
                                ep[32 * b:32 * b + 32, s, :],
                                qd2b[:, :, pi],
                                kht2[:, :, pi],
                                start=True,
                                stop=True,
                                tile_position=(0, 32 * b),
                            )
                        # dense softmax over the 256-row bank
                        ex = apool.tile([128, 32, 16], F32, tag="ex")
                        nc.scalar.activation(ex[:], ep[:], AF.Exp)
                        sm = apool.tile([128, 32], F32, tag="sm")
                        nc.vector.reduce_sum(sm[:], ex[:], axis=AX.X)
                        rcp = apool.tile([128, 32], F32, tag="rcp")
                        nc.vector.reciprocal(rcp[:], sm[:])
                        at = apool.tile([128, 32, 16], BF16, tag="at")
                        nc.vector.tensor_tensor(
                            at[:], ex[:],
                            rcp[:, :, None].to_broadcast([128, 32, 16]),
                            ALU.mult,
                        )
                        att = apool.tile([128, 512], BF16, tag="att")
                        nc.vector.transpose(
                            att[:], at[:].rearrange("p a b -> p (a b)"))
                        atts.append(att)

                if it < NP:
                    # ---- readbacks for pass `it` (issued after pass it-1's
                    # attention so the shared bufs=1 operand tiles are free;
                    # the DMAs run under Wo(it-1) + projections(it+1)) ----
                    qsrc = stage_of["q"][:].rearrange(
                        "(q d) (h pi) -> h d q pi", q=H, h=2
                    )
                    nc.sync.dma_start(qd2b[0:64, 0:16, :], qsrc[0])
                    nc.sync.dma_start(qd2b[64:128, 16:32, :], qsrc[1])
                    ksrc = stage_of["k"][:].rearrange(
                        "(q d) (h pi) -> h d q pi", q=H, h=2
                    )
                    nc.sync.dma_start(kht2[0:64, :, :], ksrc[0])
                    nc.sync.dma_start(kht2[64:128, :, :], ksrc[1])

                if it >= 1:
                    p = it - 1
                    # A @ V per bank: all four b-bands into one 4-bank
                    # psum tile, then 4 merged extraction copies
                    dstx = oft2[:].rearrange(
                        "p g (h Bk wc) -> p g h Bk wc", h=2, Bk=NBK)
                    for B in range(NBK):
                        att = atts[B]
                        avp = pav.tile([128, 4, 32, 16], F32, tag="avp")
                        for b in range(4):
                            for t in range(32):
                                nc.tensor.matmul(
                                    avp[:, b, t, :],
                                    vd[32 * b:32 * b + 32, 32 * B + t, :],
                                    att[32 * b:32 * b + 32,
                                        16 * t:16 * t + 16],
                                    start=True,
                                    stop=True,
                                    tile_position=(32 * b, 0),
                                )
                        # avp[64w+d, b, 2m+rho, q] -> oft2[64(q%2)+d, q//2,
                        #   256rho + 128B + 64w + 16b + m]
                        srcx = avp[:].rearrange(
                            "p b (m r) (g s) -> p g r (b m) s", r=2, s=2)
                        for w in range(2):
                            for sg in range(2):
                                src = srcx[64 * w:64 * w + 64, :, :, :, sg]
                                dst = dstx[64 * sg:64 * sg + 64,
                                           :, :, B, 64 * w:64 * w + 64]
                                if (w + sg + B) % 2 == 0:
                                    nc.vector.tensor_copy(dst, src)
                                else:
                                    nc.scalar.copy(dst, src)

                if it < NP:
                    # vd[32b+16w+j, 32B+2a+rho, 64w+d] =
                    #   V[r = 256rho + 128B + 8a + 4w + b, (j, d)]
                    vsrc = stage_of["v"][:].rearrange(
                        "(h B w b m) (j d) -> h w B b j m d",
                        h=2, B=NBK, w=2, b=4, m=16, j=16,
                    )
                    for w in range(2):
                        for B in range(NBK):
                            for b in range(4):
                                for rho in range(2):
                                    nc.sync.dma_start(
                                        vd[32 * b + 16 * w:
                                           32 * b + 16 * w + 16,
                                           32 * B + rho:32 * B + 32:2,
                                           64 * w:64 * w + 64],
                                        vsrc[rho, w, B, b],
                                    )


                if it >= 1:
                    p = it - 1
                    # ---- y^T = Wo^T-chunks @ oft2, + bo ----
                    for c in range(8):
                        # rotates through the ep buffers (attention is done
                        # with them by now) -> double-buffered Wo psum at no
                        # extra bank cost
                        ytp = pe_pool.tile([128, RC], F32, tag="ep")
                        for g in range(8):
                            nc.tensor.matmul(
                                ytp[:],
                                wo_sb[:, g, 128 * c:128 * c + 128],
                                oft2[:, g, :],
                                start=(g == 0),
                                stop=(g == 7),
                            )
                        ys = ypool.tile([128, RC], F32, tag="ys")
                        nc.vector.tensor_scalar(
                            ys[:], ytp[:], bo_sb[:, c:c + 1], None,
                            op0=ALU.add,
                        )
                        nc.sync.dma_start(
                            yt.rearrange("(t q) r -> q t r", q=128)[
                                :, c, p * RC:(p + 1) * RC
                            ],
                            ys[:],
                        )


    nc.finalize()
    return nc


_CACHE = {}


def _get_nc(R, RC):
    key = (R, RC)
    if key not in _CACHE:
        _CACHE[key] = build_nc(R, RC)
    return _CACHE[key]


def run_cores(x2d, Wq, Wk, Wv, Wo, bo_v, R=None, RC=512, cores=None,
              **run_kwargs):
    """x2d: (ROWS, E) fp32.  Returns (ROWS, E) fp32."""
    ROWS = x2d.shape[0]
    if cores is None:
        cores = list(range(NCORE))
    n = len(cores)
    if R is None:
        R = ROWS // n
    assert R * n == ROWS
    nc = _get_nc(R, RC)

    bf = ml_dtypes.bfloat16
    scale = 1.0 / np.sqrt(np.sqrt(float(E)))  # fold E**-0.5 into both Wq, Wk
    wq_b = (Wq.astype(np.float64) * scale).astype(bf)
    wk_b = (Wk.astype(np.float64) * scale).astype(bf)
    wv_b = Wv.astype(bf)
    wo_b = Wo.astype(bf)
    bo_in = bo_v.reshape(1, E).astype(np.float32)

    in_maps = []
    for ci in range(n):
        xs = x2d[ci * R:(ci + 1) * R].T  # (E, R)
        in_maps.append({
            "xt": np.ascontiguousarray(xs).astype(bf),
            "wq": wq_b, "wk": wk_b, "wv": wv_b, "wo": wo_b, "bo": bo_in,
        })
    res = run_bass_kernel_spmd(nc, in_maps, core_ids=cores, **run_kwargs)
    out = np.empty((ROWS, E), dtype=np.float32)
    for ci in range(n):
        ytd = res.results[ci]["yt"]  # (E, R), columns in natural row order
        out[ci * R:(ci + 1) * R] = ytd.T
    if run_kwargs.get("trace"):
        return out, res
    return out


def kernel(x, Wq, Wk, Wv, Wo, bo):
    x = np.asarray(x, dtype=np.float32)
    N, L, _ = x.shape
    y = run_cores(
        x.reshape(N * L, E),
        np.asarray(Wq, np.float32), np.asarray(Wk, np.float32),
        np.asarray(Wv, np.float32), np.asarray(Wo, np.float32),
        np.asarray(bo, np.float32),
    )
    return y.reshape(N, L, E)
